# revision 1
# baseline (speedup 1.0000x reference)
"""GAT (4-layer, softmax over dim=1) Trainium2 Bass kernel.

Sharding: data-parallel over batch B=8 -> one batch element per NeuronCore,
zero collectives. ~365 us HW exec, rel err ~2e-3 vs the jax reference.

Per-core algorithm (N=2048 nodes, F=256 features, 4 layers):
  Everything large lives in "T layout" [j on partitions, i free] so the
  reference's softmax over axis=1 (normalize over i per column j) becomes a
  free-axis reduction, and the output matmul out[i,o] = sum_j att[i,j] Wh[j,o]
  takes the attention strips directly as PE stationary operands.

  Per layer:
    Wh = hT.T @ W (PE, fp16 operands); f1/f2 = h @ (W a1|2) (PE, tiny)
    f1 pre-broadcast across partitions via a host-built [F,128] wa1-column
    matrix: f1b = wa1b.T @ hT (PE), skipping a row->partition round trip.
    per j-strip [128 x 2048], software-pipelined (post-exp work lags 2):
      pm   = f1b + maskT_strip     (DVE; bf16 additive mask {0,-500})
      pm   = Prelu(pm + f2[j])     (ACT, alpha=0.2, in place; -500 -> -100)
      expe = Exp(pm) -> fp16, accum_out -> s[j]  (ACT; shift-free softmax --
             e is bounded ~ +/-5 here, and masked entries underflow to 0)
      r = 1/s (DVE); Whs[j,:] = Wh[j,:] * r[j] -> fp16 (DVE)
      8 PE matmuls accumulate outT[o,i] += Whs.T @ expe into 8 PSUM banks
    tail: hT_next = Prelu(outT psum) on ACT, already in [f,n] layout.
  Final layer: same outT accumulation, then PE identity-matmul transposes
  back to natural [i,o] and DMA out (fp32 end to end after PSUM).

Host prep: transposed fp16 x, transposed bf16 additive adjacency mask
(adj is shared by all 4 layers; {0,-500} is exact through prelu+exp),
fp16 W, W@a1/W@a2, and the wa1 broadcast matrix.

Notes for this container: walrus accepts at most one sync-wait per
instruction (split_multi_waits hoists extras onto EventSemaphore insts);
custom DVE ops and tensor_tensor_reduce do not compile ("ISA wrong length");
ACT Lrelu ignores its alpha operand but Prelu honors it; fp32 matmuls run
at roughly quarter bf16/fp16 rate, which is why PE operands are fp16.
"""

import numpy as np
import ml_dtypes

import bass_rust
import concourse.bass as bass
import concourse.mybir as mybir
import concourse.tile as tile
from concourse.bass_utils import run_bass_kernel_spmd

f32 = mybir.dt.float32
bf16 = mybir.dt.bfloat16
f16 = mybir.dt.float16
AFT = mybir.ActivationFunctionType

B, N, F, L = 8, 2048, 256, 4
NT = N // 128  # 16 node tiles
FC = F // 128  # 2 feature chunks
IC = N // 512  # 4 i-chunks per strip
ALPHA = 0.2
MASKADD = -500.0


def split_multi_waits(nc):
    """This container's walrus supports at most one sync-wait per instruction;
    Tile's exit drain (and occasionally the scheduler) attaches several. Hoist
    extras onto same-engine EventSemaphore instructions placed just before."""
    for fn in nc.m.functions:
        for blk in fn.blocks:
            new_list, changed = [], False
            for inst in blk.instructions:
                si = inst.sync_info
                if si is not None and len(si.on_wait) > 1:
                    waits = list(si.on_wait)
                    for k, w in enumerate(waits[:-1]):
                        es = mybir.InstEventSemaphore(name=f"{inst.name}_wsplit{k}")
                        es.engine = inst.engine
                        es.sync_info = bass_rust.SyncInfo(on_wait=[w], on_update=[])
                        new_list.append(es)
                    si.on_wait = [waits[-1]]
                    changed = True
                new_list.append(inst)
            if changed:
                blk.instructions = new_list


def build_nc(do_split=True):
    nc = bass.Bass()
    xT_d = nc.dram_tensor("xT", [F, N], f16, kind="ExternalInput")
    mask_d = nc.dram_tensor("maskT", [N, N], bf16, kind="ExternalInput")
    W_d = nc.dram_tensor("W", [L, F, F], f16, kind="ExternalInput")
    wa_d = nc.dram_tensor("wa", [L, F, 2], f16, kind="ExternalInput")
    wab_d = nc.dram_tensor("wab", [L, F, 128], f16, kind="ExternalInput")
    ones_d = nc.dram_tensor("ones", [1, 128], f32, kind="ExternalInput")
    ident_d = nc.dram_tensor("ident", [128, 128], f32, kind="ExternalInput")
    out_d = nc.dram_tensor("out", [N, F], f32, kind="ExternalOutput")

    with tile.TileContext(nc) as tc:
        with (
            tc.tile_pool(name="const", bufs=1) as constp,
            tc.tile_pool(name="hT", bufs=2) as hTp,
            tc.tile_pool(name="wl", bufs=2) as wlp,
            tc.tile_pool(name="wh", bufs=1) as whp,
            tc.tile_pool(name="fvec", bufs=2) as fvp,
            tc.tile_pool(name="strip", bufs=3) as stripp,
            tc.tile_pool(name="sr", bufs=4) as srp,
            tc.tile_pool(name="whs", bufs=3) as whsp,
            tc.tile_pool(name="outsb", bufs=3) as outp,
            tc.tile_pool(name="bank", bufs=8, space="PSUM") as psp,
        ):
            ones_sb = constp.tile([1, 128], f32)
            nc.sync.dma_start(ones_sb[:], ones_d[:])
            ident_sb = constp.tile([128, 128], f32)
            nc.sync.dma_start(ident_sb[:], ident_d[:])
            hT_cur = hTp.tile([128, FC * N], f16, tag="hT")
            for fc in range(FC):
                nc.sync.dma_start(
                    hT_cur[:, fc * N : (fc + 1) * N],
                    xT_d[fc * 128 : (fc + 1) * 128, :],
                )
            def load_layer_weights(l):
                W_sb = wlp.tile([128, FC * F], f16, tag="W", name=f"W_{l}")
                wa_sb = wlp.tile([128, FC * 2], f16, tag="wa", name=f"wa_{l}")
                wab_sb = wlp.tile([128, FC * 128], f16, tag="wab", name=f"wab_{l}")
                for fc in range(FC):
                    nc.sync.dma_start(
                        W_sb[:, fc * F : (fc + 1) * F],
                        W_d[l, fc * 128 : (fc + 1) * 128, :],
                    )
                    nc.sync.dma_start(
                        wa_sb[:, fc * 2 : (fc + 1) * 2],
                        wa_d[l, fc * 128 : (fc + 1) * 128, :],
                    )
                    nc.sync.dma_start(
                        wab_sb[:, fc * 128 : (fc + 1) * 128],
                        wab_d[l, fc * 128 : (fc + 1) * 128, :],
                    )
                return W_sb, wa_sb, wab_sb

            weights0 = load_layer_weights(0)
            mask_sb = constp.tile([128, NT * N], bf16)
            for jt in range(NT):
                nc.sync.dma_start(
                    mask_sb[:, jt * N : (jt + 1) * N],
                    mask_d[jt * 128 : (jt + 1) * 128, :],
                )

            for l in range(L):
                if l == 0:
                    W_sb, wa_sb, wab_sb = weights0
                else:
                    W_sb, wa_sb, wab_sb = load_layer_weights(l)

                # ---- f-phase ----
                f12_sb = fvp.tile([128, NT * 2], f32, tag="f12")
                for nt in range(NT):
                    ps = psp.tile([128, 512], f32, tag="bank")
                    for fc in range(FC):
                        nc.tensor.matmul(
                            ps[:, 0:2],
                            hT_cur[:, fc * N + nt * 128 : fc * N + (nt + 1) * 128],
                            wa_sb[:, fc * 2 : (fc + 1) * 2],
                            start=(fc == 0),
                            stop=(fc == FC - 1),
                        )
                    nc.vector.tensor_copy(f12_sb[:, nt * 2 : nt * 2 + 2], ps[:, 0:2])

                f1b = fvp.tile([128, N], f32, tag="f1b")
                for ic in range(IC):
                    ps = psp.tile([128, 512], f32, tag="bank")
                    for fc in range(FC):
                        nc.tensor.matmul(
                            ps[:, :],
                            wab_sb[:, fc * 128 : (fc + 1) * 128],
                            hT_cur[:, fc * N + ic * 512 : fc * N + (ic + 1) * 512],
                            start=(fc == 0),
                            stop=(fc == FC - 1),
                        )
                    nc.vector.tensor_copy(f1b[:, ic * 512 : (ic + 1) * 512], ps[:, :])

                Wh_sb = whp.tile([128, NT * F], f16, tag="Wh")
                for nt in range(NT):
                    ps = psp.tile([128, 512], f32, tag="bank")
                    for fc in range(FC):
                        nc.tensor.matmul(
                            ps[:, 0:F],
                            hT_cur[:, fc * N + nt * 128 : fc * N + (nt + 1) * 128],
                            W_sb[:, fc * F : (fc + 1) * F],
                            start=(fc == 0),
                            stop=(fc == FC - 1),
                        )
                    if nt % 2 == 0:
                        nc.scalar.copy(Wh_sb[:, nt * F : (nt + 1) * F], ps[:, 0:F])
                    else:
                        nc.vector.tensor_copy(
                            Wh_sb[:, nt * F : (nt + 1) * F], ps[:, 0:F]
                        )

                # ---- strip loop (software-pipelined; lag-1 for post-exp ops
                # so DVE's in-order stream never blocks the next pm) ----
                psum_out = [
                    psp.tile([128, 512], f32, tag="bank", name=f"po_{l}_{k}")
                    for k in range(8)
                ]
                expe_t = [None] * NT
                s_t = [None] * NT

                def emit_front(jt):
                    pm = stripp.tile([128, N], f32, tag="pm", name=f"pm_{l}_{jt}", bufs=2)
                    nc.vector.tensor_tensor(
                        pm[:, :],
                        f1b[:, :],
                        mask_sb[:, jt * N : (jt + 1) * N],
                        mybir.AluOpType.add,
                    )
                    nc.scalar.activation(
                        pm[:, :],
                        pm[:, :],
                        AFT.Prelu,
                        bias=f12_sb[:, jt * 2 + 1 : jt * 2 + 2],
                        scale=1.0,
                        alpha=ALPHA,
                    )
                    expe = stripp.tile([128, N], f16, tag="expe", name=f"ex_{l}_{jt}")
                    s = srp.tile([128, 1], f32, tag="s", name=f"s_{l}_{jt}")
                    nc.scalar.activation(
                        expe[:, :], pm[:, :], AFT.Exp, accum_out=s[:, :]
                    )
                    expe_t[jt] = expe
                    s_t[jt] = s

                def emit_back(jt):
                    r_t = srp.tile([128, 1], f32, tag="r", name=f"r_{l}_{jt}")
                    nc.vector.reciprocal(r_t[:, :], s_t[jt][:, :])
                    whs_t = whsp.tile([128, F], f16, tag="whs", name=f"wh_{l}_{jt}")
                    nc.vector.tensor_scalar_mul(
                        whs_t[:, :], Wh_sb[:, jt * F : (jt + 1) * F], r_t[:, :]
                    )
                    for oc in range(2):
                        for ic in range(IC):
                            nc.tensor.matmul(
                                psum_out[oc * IC + ic][:, :],
                                whs_t[:, oc * 128 : (oc + 1) * 128],
                                expe_t[jt][:, ic * 512 : (ic + 1) * 512],
                                start=(jt == 0),
                                stop=(jt == NT - 1),
                            )

                for jt in range(NT + 2):
                    if jt >= 2:
                        emit_back(jt - 2)
                    if jt < NT:
                        emit_front(jt)

                # ---- tail ----
                if l < L - 1:
                    hT_next = hTp.tile([128, FC * N], f16, tag="hT")
                else:
                    hT_next = hTp.tile([128, FC * N], f32, tag="hTf32", bufs=1)
                for ic in range(IC):
                    for oc in range(2):
                        dst = hT_next[:, oc * N + ic * 512 : oc * N + (ic + 1) * 512]
                        ps = psum_out[oc * IC + ic]
                        nc.scalar.activation(dst, ps[:, :], AFT.Prelu, alpha=ALPHA)
                if l < L - 1:
                    hT_cur = hT_next
                else:
                    # transpose houtT [o, i] -> out [i, o] via PE identity matmuls
                    for nt in range(NT):
                        ob = outp.tile([128, F], f32, tag="ob")
                        for oc in range(FC):
                            pst = psp.tile([128, 512], f32, tag="bank", name=f"tr_{nt}_{oc}")
                            nc.tensor.matmul(
                                pst[:, 0:128],
                                hT_next[:, oc * N + nt * 128 : oc * N + (nt + 1) * 128],
                                ident_sb[:, :],
                                start=True,
                                stop=True,
                            )
                            if oc % 2 == 0:
                                nc.scalar.copy(
                                    ob[:, oc * 128 : (oc + 1) * 128], pst[:, 0:128]
                                )
                            else:
                                nc.vector.tensor_copy(
                                    ob[:, oc * 128 : (oc + 1) * 128], pst[:, 0:128]
                                )
                        nc.sync.dma_start(out_d[nt * 128 : (nt + 1) * 128, :], ob[:, :])

    if do_split:
        split_multi_waits(nc)
    return nc


_NC = None


def _get_nc():
    global _NC
    if _NC is None:
        _NC = build_nc()
    return _NC


def _host_prep(x, adj, W0, Wrest, A):
    x = np.asarray(x, dtype=np.float32)
    adj = np.asarray(adj)
    W_all = np.stack(
        [np.asarray(W0, dtype=np.float32)]
        + [np.asarray(Wrest[i], dtype=np.float32) for i in range(L - 1)]
    )  # [4, F, F]
    A = np.asarray(A, dtype=np.float32)
    wa = np.empty((L, F, 2), dtype=np.float32)
    for l in range(L):
        wa[l, :, 0] = W_all[l] @ A[l, :F]
        wa[l, :, 1] = W_all[l] @ A[l, F:]
    ones = np.ones((1, 128), dtype=np.float32)
    ident = np.eye(128, dtype=np.float32)
    W_16 = W_all.astype(np.float16)
    wa_16 = wa.astype(np.float16)
    wab_16 = np.repeat(wa_16[:, :, 0:1], 128, axis=2)

    in_maps = []
    for b in range(B):
        xT = np.ascontiguousarray(x[b].T).astype(np.float16)
        adjT = adj[b].T.astype(np.float32)
        maskT = ((adjT - 1.0) * (-MASKADD)).astype(ml_dtypes.bfloat16)
        in_maps.append(
            {
                "xT": xT,
                "maskT": maskT,
                "W": W_16,
                "wa": wa_16,
                "wab": wab_16,
                "ones": ones,
                "ident": ident,
            }
        )
    return in_maps


def kernel(x, adj, W0, Wrest, A, _trace=False, _trace_kwargs=None):
    nc = _get_nc()
    in_maps = _host_prep(x, adj, W0, Wrest, A)
    res = run_bass_kernel_spmd(
        nc,
        in_maps,
        core_ids=list(range(B)),
        trace=_trace,
        **(_trace_kwargs or {}),
    )
    out = np.stack([res.results[b]["out"] for b in range(B)])
    if _trace:
        kernel.last_exec_time_ns = res.exec_time_ns
        kernel.last_results = res
    return out



# revision 10
# speedup vs baseline: 1.0294x; 1.0294x over previous
"""GAT (4-layer, softmax over dim=1) Trainium2 Bass kernel, v2.

Sharding: data-parallel over batch B=8 -> one batch element per NeuronCore,
zero collectives.

Per-core algorithm (N=2048 nodes, F=256 features, 4 layers), all in
"T layout" [j on partitions, i free] so softmax over axis=1 is a free-axis
reduction and attention strips feed PE directly:

  Per layer:
    f2[n]  : per-node attention bias, PE matmuls (1 col per node tile)
    f1b    : f1 broadcast across partitions via host-built wab matrix (PE)
    Wh     : h @ W (PE, fp16)
    strip loop over 16 j-strips, software pipelined (back lags front by 2):
      front: t = f1b + maskT_strip          (DVE TT fp16, 2x mode)
             leakyrelu+bias by route:
               A: ACT Prelu(t + f2[j])            (1 ACT pass)
               B: z=(t+f2); w=(0.2t+0.2f2);       (2 DVE TS passes, 4x mode)
                  l=max(z,w)                      (1 DVE TT pass)
               G: z=(t+f2) via GPSIMD STT; l=(0.2z max z) via GPSIMD STT
             expe = Exp(l) -> fp16, accum_out -> s[j]   (ACT)
      back:  r = 1/s (DVE, per strip-pair); whs = Wh[j]*r (DVE TS)
             8 PE matmuls accumulate outT[o,i] += whs.T @ expe (8 PSUM banks)
    tail: hT_next = Prelu(outT psum), split across ACT/GPSIMD/DVE.
  Final layer writes hT (fp16, [o,i]) straight to DRAM; the host transposes
  to [i,o] (no PE identity-transpose pass).

The route mix (A/B/G) load-balances the leakyrelu work across ACT, DVE and
GPSIMD so ACT only pays the irreducible exp pass for most strips.

Notes for this container: walrus accepts at most one sync-wait per
instruction (split_multi_waits hoists extras onto EventSemaphore insts);
scalar_tensor_tensor runs at 1x on DVE (use GPSIMD for it); fp8 matmuls
lose too much precision through 4 layers (tested); ACT Lrelu ignores its
alpha operand but Prelu honors it.
"""

import numpy as np

import bass_rust
import concourse.bass as bass
import concourse.mybir as mybir
import concourse.tile as tile
from concourse.bass_utils import run_bass_kernel_spmd

f32 = mybir.dt.float32
f16 = mybir.dt.float16
AFT = mybir.ActivationFunctionType
ALU = mybir.AluOpType

B, N, F, L = 8, 2048, 256, 4
NT = N // 128  # 16 node tiles (j strips)
FC = F // 128  # 2 feature chunks
IC = N // 512  # 4 i-chunks per strip
ALPHA = 0.2
MASKADD = -500.0

# route per j-strip: 'A' = ACT prelu; 'B' = DVE max-form; 'L' = DVE max-form
# with the final max on GPSIMD (Pool supports plain tensor_tensor only)
ROUTES = list("BBBA" "BBBA" "BBBA" "BBBA")
# tail prelu engine per chunk (8 chunks): a=ACT, g=GPSIMD, v=DVE
TAIL = list("aaaa" "aaaa")
# Wh psum->sbuf copy engine per node-tile pair (8): a=ACT, v=DVE, g=GPSIMD
WHCP = list("avav" "avav")
# f1b psum->sbuf copy engine per chunk (4)
F1CP = list("aaaa")


def split_multi_waits(nc):
    """This container's walrus supports at most one sync-wait per instruction;
    Tile's exit drain (and occasionally the scheduler) attaches several. Hoist
    extras onto same-engine EventSemaphore instructions placed just before."""
    for fn in nc.m.functions:
        for blk in fn.blocks:
            new_list, changed = [], False
            for inst in blk.instructions:
                si = inst.sync_info
                if si is not None and len(si.on_wait) > 1:
                    waits = list(si.on_wait)
                    for k, w in enumerate(waits[:-1]):
                        es = mybir.InstEventSemaphore(name=f"{inst.name}_wsplit{k}")
                        es.engine = inst.engine
                        es.sync_info = bass_rust.SyncInfo(on_wait=[w], on_update=[])
                        new_list.append(es)
                    si.on_wait = [waits[-1]]
                    changed = True
                new_list.append(inst)
            if changed:
                blk.instructions = new_list


def build_nc(do_split=True):
    nc = bass.Bass()
    xT_d = nc.dram_tensor("xT", [F, N], f16, kind="ExternalInput")
    mask_d = nc.dram_tensor("maskT", [N, N], f16, kind="ExternalInput")
    W_d = nc.dram_tensor("W", [L, F, F], f16, kind="ExternalInput")
    wa2_d = nc.dram_tensor("wa2", [L, F, 1], f16, kind="ExternalInput")
    wab_d = nc.dram_tensor("wab", [L, F, 128], f16, kind="ExternalInput")
    outT_d = nc.dram_tensor("outT", [F, N], f16, kind="ExternalOutput")

    with tile.TileContext(nc) as tc:
        with (
            tc.tile_pool(name="const", bufs=1) as constp,
            tc.tile_pool(name="hT", bufs=2) as hTp,
            tc.tile_pool(name="wl", bufs=2) as wlp,
            tc.tile_pool(name="wh", bufs=1) as whp,
            tc.tile_pool(name="fvec", bufs=2) as fvp,
            tc.tile_pool(name="strip", bufs=3) as stripp,
            tc.tile_pool(name="zw", bufs=2) as zwp,
            tc.tile_pool(name="sr", bufs=4) as srp,
            tc.tile_pool(name="whs", bufs=3) as whsp,
            tc.tile_pool(name="bank", bufs=8, space="PSUM") as psp,
        ):
            hT_cur = hTp.tile([128, FC * N], f16, tag="hT")
            for fc in range(FC):
                nc.sync.dma_start(
                    hT_cur[:, fc * N : (fc + 1) * N],
                    xT_d[fc * 128 : (fc + 1) * 128, :],
                )

            def load_layer_weights(l):
                W_sb = wlp.tile([128, FC * F], f16, tag="W", name=f"W_{l}")
                wa2_sb = wlp.tile([128, FC * 1], f16, tag="wa2", name=f"wa2_{l}")
                wab_sb = wlp.tile([128, FC * 128], f16, tag="wab", name=f"wab_{l}")
                for fc in range(FC):
                    nc.sync.dma_start(
                        W_sb[:, fc * F : (fc + 1) * F],
                        W_d[l, fc * 128 : (fc + 1) * 128, :],
                    )
                    nc.sync.dma_start(
                        wa2_sb[:, fc : fc + 1],
                        wa2_d[l, fc * 128 : (fc + 1) * 128, :],
                    )
                    nc.sync.dma_start(
                        wab_sb[:, fc * 128 : (fc + 1) * 128],
                        wab_d[l, fc * 128 : (fc + 1) * 128, :],
                    )
                return W_sb, wa2_sb, wab_sb

            weights0 = load_layer_weights(0)
            mask_sb = constp.tile([128, NT * N], f16)
            for jt in range(NT):
                nc.sync.dma_start(
                    mask_sb[:, jt * N : (jt + 1) * N],
                    mask_d[jt * 128 : (jt + 1) * 128, :],
                )

            for l in range(L):
                if l == 0:
                    W_sb, wa2_sb, wab_sb = weights0
                else:
                    W_sb, wa2_sb, wab_sb = load_layer_weights(l)

                # ---- f1 broadcast across partitions (gates strip 0) ----
                f1b = fvp.tile([128, N], f16, tag="f1b")
                for ic in range(IC):
                    ps = psp.tile([128, 512], f32, tag="bank", name=f"f1b_{l}_{ic}")
                    for fc in range(FC):
                        nc.tensor.matmul(
                            ps[:, :],
                            wab_sb[:, fc * 128 : (fc + 1) * 128],
                            hT_cur[:, fc * N + ic * 512 : fc * N + (ic + 1) * 512],
                            start=(fc == 0),
                            stop=(fc == FC - 1),
                        )
                    dst = f1b[:, ic * 512 : (ic + 1) * 512]
                    if F1CP[ic] == "a":
                        nc.scalar.copy(dst, ps[:, :])
                    elif F1CP[ic] == "g":
                        nc.gpsimd.tensor_copy(dst, ps[:, :])
                    else:
                        nc.vector.tensor_copy(dst, ps[:, :])

                # ---- f2 (one column per node tile, all in one PSUM bank) ----
                f2_sb = fvp.tile([128, NT], f32, tag="f2")
                f2s_sb = fvp.tile([128, NT], f32, tag="f2s")
                ps_f2 = psp.tile([128, 512], f32, tag="bank", name=f"f2_{l}")
                for nt in range(NT):
                    for fc in range(FC):
                        nc.tensor.matmul(
                            ps_f2[:, nt : nt + 1],
                            hT_cur[:, fc * N + nt * 128 : fc * N + (nt + 1) * 128],
                            wa2_sb[:, fc : fc + 1],
                            start=(fc == 0),
                            stop=(fc == FC - 1),
                        )
                nc.vector.tensor_copy(f2_sb[:, :], ps_f2[:, 0:NT])
                # f2n = -0.8*f2: max(t, 0.2t + f2n) + f2 == leakyrelu(t + f2)
                nc.vector.tensor_scalar(
                    f2s_sb[:, :], f2_sb[:, :], ALPHA - 1.0, None, ALU.mult
                )

                # ---- Wh (two node tiles share a PSUM bank and one copy) ----
                Wh_sb = whp.tile([128, NT * F], f16, tag="Wh")
                for np_ in range(NT // 2):
                    ps = psp.tile([128, 512], f32, tag="bank", name=f"wh_{l}_{np_}")
                    for half in range(2):
                        nt = np_ * 2 + half
                        for fc in range(FC):
                            nc.tensor.matmul(
                                ps[:, half * F : (half + 1) * F],
                                hT_cur[:, fc * N + nt * 128 : fc * N + (nt + 1) * 128],
                                W_sb[:, fc * F : (fc + 1) * F],
                                start=(fc == 0),
                                stop=(fc == FC - 1),
                            )
                    dst = Wh_sb[:, np_ * 512 : (np_ + 1) * 512]
                    if WHCP[np_] == "a":
                        nc.scalar.copy(dst, ps[:, :])
                    elif WHCP[np_] == "g":
                        nc.gpsimd.tensor_copy(dst, ps[:, :])
                    else:
                        nc.vector.tensor_copy(dst, ps[:, :])

                # ---- strip loop (pipelined, back lags front by 2) ----
                psum_out = [
                    psp.tile([128, 512], f32, tag="bank", name=f"po_{l}_{k}")
                    for k in range(8)
                ]
                expe_t = [None] * NT
                s_t = [None] * (NT // 2)
                r_t = [None] * (NT // 2)

                def emit_front(jt):
                    t = stripp.tile([128, N], f16, tag="t", name=f"t_{l}_{jt}")
                    nc.vector.tensor_tensor(
                        t[:, :],
                        f1b[:, :],
                        mask_sb[:, jt * N : (jt + 1) * N],
                        ALU.add,
                    )
                    if jt % 2 == 0:
                        s_t[jt // 2] = srp.tile(
                            [128, 2], f32, tag="s", name=f"s_{l}_{jt}"
                        )
                    s_col = s_t[jt // 2][:, jt % 2 : jt % 2 + 1]
                    f2c = f2_sb[:, jt : jt + 1]
                    route = ROUTES[jt]
                    exp_bias = 0.0
                    if route == "A":
                        nc.scalar.activation(
                            t[:, :], t[:, :], AFT.Prelu,
                            bias=f2c, scale=1.0, alpha=ALPHA,
                        )
                        l_in = t
                    else:  # B / L: m = max(t, 0.2t - 0.8 f2); exp adds f2 bias
                        w = zwp.tile([128, N], f16, tag="w", name=f"w_{l}_{jt}")
                        nc.vector.tensor_scalar(
                            w[:, :], t[:, :], ALPHA, f2s_sb[:, jt : jt + 1],
                            ALU.mult, ALU.add,
                        )
                        nc.vector.tensor_tensor(t[:, :], t[:, :], w[:, :], ALU.max)
                        l_in = t
                        exp_bias = f2c
                    expe = stripp.tile([128, N], f16, tag="expe", name=f"ex_{l}_{jt}")
                    nc.scalar.activation(
                        expe[:, :], l_in[:, :], AFT.Exp, bias=exp_bias,
                        accum_out=s_col,
                    )
                    expe_t[jt] = expe

                def emit_back(jt):
                    if jt % 2 == 0:
                        r_t[jt // 2] = srp.tile(
                            [128, 2], f32, tag="r", name=f"r_{l}_{jt}"
                        )
                        nc.vector.reciprocal(r_t[jt // 2][:, :], s_t[jt // 2][:, :])
                    r_col = r_t[jt // 2][:, jt % 2 : jt % 2 + 1]
                    whs_t = whsp.tile([128, F], f16, tag="whs", name=f"wh_{l}_{jt}")
                    nc.vector.tensor_scalar_mul(
                        whs_t[:, :], Wh_sb[:, jt * F : (jt + 1) * F], r_col
                    )
                    for oc in range(2):
                        for ic in range(IC):
                            nc.tensor.matmul(
                                psum_out[oc * IC + ic][:, :],
                                whs_t[:, oc * 128 : (oc + 1) * 128],
                                expe_t[jt][:, ic * 512 : (ic + 1) * 512],
                                start=(jt == 0),
                                stop=(jt == NT - 1),
                            )

                # back(jt) needs front(jt+1) done (paired reciprocal), so the
                # lag of 2 keeps the pairing legal.
                for jt in range(NT + 2):
                    if jt >= 2:
                        emit_back(jt - 2)
                    if jt < NT:
                        emit_front(jt)

                # ---- tail: prelu psum -> next hT (fp16) ----
                hT_next = hTp.tile([128, FC * N], f16, tag="hT", name=f"hTn_{l}")
                for ic in range(IC):
                    for oc in range(2):
                        k = oc * IC + ic
                        dst = hT_next[:, oc * N + ic * 512 : oc * N + (ic + 1) * 512]
                        ps = psum_out[k]
                        eng = TAIL[k]
                        if eng == "a":
                            nc.scalar.activation(dst, ps[:, :], AFT.Prelu, alpha=ALPHA)
                        elif eng == "g":
                            nc.gpsimd.scalar_tensor_tensor(
                                dst, ps[:, :], ALPHA, ps[:, :], ALU.mult, ALU.max
                            )
                        else:
                            z = zwp.tile([128, 512], f16, tag="tl", name=f"tl_{l}_{k}")
                            nc.vector.tensor_scalar(
                                z[:, :], ps[:, :], ALPHA, None, ALU.mult
                            )
                            nc.vector.tensor_tensor(dst, ps[:, :], z[:, :], ALU.max)
                if l < L - 1:
                    hT_cur = hT_next
                else:
                    for oc in range(FC):
                        nc.sync.dma_start(
                            outT_d[oc * 128 : (oc + 1) * 128, :],
                            hT_next[:, oc * N : (oc + 1) * N],
                        )

    if do_split:
        split_multi_waits(nc)
    return nc


_NC = None


def _get_nc():
    global _NC
    if _NC is None:
        _NC = build_nc()
    return _NC


def _host_prep(x, adj, W0, Wrest, A):
    x = np.asarray(x, dtype=np.float32)
    adj = np.asarray(adj)
    W_all = np.stack(
        [np.asarray(W0, dtype=np.float32)]
        + [np.asarray(Wrest[i], dtype=np.float32) for i in range(L - 1)]
    )  # [4, F, F]
    A = np.asarray(A, dtype=np.float32)
    wa1 = np.empty((L, F), dtype=np.float32)
    wa2 = np.empty((L, F, 1), dtype=np.float32)
    for l in range(L):
        wa1[l] = W_all[l] @ A[l, :F]
        wa2[l, :, 0] = W_all[l] @ A[l, F:]
    W_16 = W_all.astype(np.float16)
    wa2_16 = wa2.astype(np.float16)
    wab_16 = np.repeat(wa1.astype(np.float16)[:, :, None], 128, axis=2)

    in_maps = []
    for b in range(B):
        xT = np.ascontiguousarray(x[b].T).astype(np.float16)
        maskT = ((adj[b].T == 0) * np.float32(MASKADD)).astype(np.float16)
        in_maps.append(
            {
                "xT": xT,
                "maskT": maskT,
                "W": W_16,
                "wa2": wa2_16,
                "wab": wab_16,
            }
        )
    return in_maps


def kernel(x, adj, W0, Wrest, A, _trace=False, _trace_kwargs=None):
    nc = _get_nc()
    in_maps = _host_prep(x, adj, W0, Wrest, A)
    res = run_bass_kernel_spmd(
        nc,
        in_maps,
        core_ids=list(range(B)),
        trace=_trace,
        **(_trace_kwargs or {}),
    )
    out = np.stack(
        [
            np.ascontiguousarray(res.results[b]["outT"].astype(np.float32).T)
            for b in range(B)
        ]
    )
    if _trace:
        kernel.last_exec_time_ns = res.exec_time_ns
        kernel.last_results = res
    return out


# revision 11
# speedup vs baseline: 1.2956x; 1.2586x over previous
"""GAT (4-layer, softmax over dim=1) Trainium2 Bass kernel, v3.

Sharding: data-parallel over batch B=8 -> one batch element per NeuronCore,
zero collectives.

Per-core algorithm (N=2048 nodes, F=256 features, 4 layers), all in
"T layout" [j on partitions, i free] so softmax over axis=1 is a free-axis
reduction and attention strips feed PE directly:

  Per layer:
    f1b    : f1 broadcast across partitions via host-built wab matrix (PE);
             psum->sbuf copies on DVE (idle during the boundary)
    f2[n]  : per-node bias, PE 1-col matmuls into one PSUM bank, copied out
             in quarters so early strips unblock fast
    Wh     : h @ W (PE, fp16), two node tiles per PSUM bank
    strip loop over 16 j-strips, pipelined (paired backs lag fronts by 2):
      front: t = f1b + maskT (DVE TT fp16 2x, one op per strip PAIR via a
             stride-0 broadcast AP on f1b)
             route A: ACT Prelu(t + f2[j]) then Exp
             route B: w = 0.2t - 0.8 f2[j] (DVE TS 2-op, 4x);
                      t = max(t, w) (DVE TT); Exp(t + f2[j]) via exp bias
             expe = Exp -> fp16, accum_out -> s[j] column (ACT)
      back (per pair): r = 1/s (DVE), whs = Wh[j]*r (DVE TS) x2,
             16 consecutive PE matmuls (keeps PE p-state ramped)
    tail: hT_next = Prelu(outT psum) on ACT (fills the boundary bubble).
  Final layer writes hT (fp16, [o,i]) straight to DRAM; the host transposes
  to [i,o] (no PE identity-transpose pass).

Notes for this container: walrus accepts at most one sync-wait per
instruction (split_multi_waits hoists extras); GPSIMD/Pool rejects all
generic elementwise ops at ISA level (TensorTensor, TensorScalarPtr) so
only ACT+DVE carry elementwise work; scalar_tensor_tensor is 1x on DVE;
fp8 matmuls lose too much precision through 4 layers (tested); engine op
costs inflate ~20% under concurrency vs isolated microbenchmarks.
"""

import numpy as np

import bass_rust
import concourse.bass as bass
import concourse.mybir as mybir
import concourse.tile as tile
from concourse.bass_utils import run_bass_kernel_spmd

f32 = mybir.dt.float32
f16 = mybir.dt.float16
AFT = mybir.ActivationFunctionType
ALU = mybir.AluOpType

B, N, F, L = 8, 2048, 256, 4
NT = N // 128  # 16 node tiles (j strips)
FC = F // 128  # 2 feature chunks
IC = N // 512  # 4 i-chunks per strip
ALPHA = 0.2
MASKADD = -500.0

# route per j-strip: 'A' = ACT prelu, 'B' = DVE max-form
ROUTES = list("BBBB" "BABA" "BABA" "BBAB")
# Wh psum->sbuf copy engine per node-tile pair (8): a=ACT, v=DVE
WHCP = list("avav" "avav")


def split_multi_waits(nc):
    """This container's walrus supports at most one sync-wait per instruction;
    Tile's exit drain (and occasionally the scheduler) attaches several. Hoist
    extras onto same-engine EventSemaphore instructions placed just before."""
    for fn in nc.m.functions:
        for blk in fn.blocks:
            new_list, changed = [], False
            for inst in blk.instructions:
                si = inst.sync_info
                if si is not None and len(si.on_wait) > 1:
                    waits = list(si.on_wait)
                    for k, w in enumerate(waits[:-1]):
                        es = mybir.InstEventSemaphore(name=f"{inst.name}_wsplit{k}")
                        es.engine = inst.engine
                        es.sync_info = bass_rust.SyncInfo(on_wait=[w], on_update=[])
                        new_list.append(es)
                    si.on_wait = [waits[-1]]
                    changed = True
                new_list.append(inst)
            if changed:
                blk.instructions = new_list


def build_nc(do_split=True):
    nc = bass.Bass()
    xT_d = nc.dram_tensor("xT", [F, N], f16, kind="ExternalInput")
    mask_d = nc.dram_tensor("maskT", [N, N], f16, kind="ExternalInput")
    W_d = nc.dram_tensor("W", [L, F, F], f16, kind="ExternalInput")
    wa2_d = nc.dram_tensor("wa2", [L, F, 1], f16, kind="ExternalInput")
    wab_d = nc.dram_tensor("wab", [L, F, 128], f16, kind="ExternalInput")
    outT_d = nc.dram_tensor("outT", [F, N], f16, kind="ExternalOutput")

    with tile.TileContext(nc) as tc:
        with (
            tc.tile_pool(name="const", bufs=1) as constp,
            tc.tile_pool(name="hT", bufs=2) as hTp,
            tc.tile_pool(name="wl", bufs=2) as wlp,
            tc.tile_pool(name="wh", bufs=1) as whp,
            tc.tile_pool(name="fvec", bufs=2) as fvp,
            tc.tile_pool(name="tpair", bufs=2) as tpp,
            tc.tile_pool(name="expe", bufs=4) as expp,
            tc.tile_pool(name="zw", bufs=2) as zwp,
            tc.tile_pool(name="sr", bufs=4) as srp,
            tc.tile_pool(name="whs", bufs=4) as whsp,
            tc.tile_pool(name="bank", bufs=8, space="PSUM") as psp,
        ):
            hT_cur = hTp.tile([128, FC * N], f16, tag="hT")
            for fc in range(FC):
                nc.sync.dma_start(
                    hT_cur[:, fc * N : (fc + 1) * N],
                    xT_d[fc * 128 : (fc + 1) * 128, :],
                )

            def load_layer_weights(l):
                W_sb = wlp.tile([128, FC * F], f16, tag="W", name=f"W_{l}")
                wa2_sb = wlp.tile([128, FC * 1], f16, tag="wa2", name=f"wa2_{l}")
                wab_sb = wlp.tile([128, FC * 128], f16, tag="wab", name=f"wab_{l}")
                for fc in range(FC):
                    nc.sync.dma_start(
                        W_sb[:, fc * F : (fc + 1) * F],
                        W_d[l, fc * 128 : (fc + 1) * 128, :],
                    )
                    nc.sync.dma_start(
                        wa2_sb[:, fc : fc + 1],
                        wa2_d[l, fc * 128 : (fc + 1) * 128, :],
                    )
                    nc.sync.dma_start(
                        wab_sb[:, fc * 128 : (fc + 1) * 128],
                        wab_d[l, fc * 128 : (fc + 1) * 128, :],
                    )
                return W_sb, wa2_sb, wab_sb

            weights0 = load_layer_weights(0)
            mask_sb = constp.tile([128, NT, N], f16)
            for jt in range(NT):
                nc.sync.dma_start(
                    mask_sb[:, jt, :],
                    mask_d[jt * 128 : (jt + 1) * 128, :],
                )

            for l in range(L):
                if l == 0:
                    W_sb, wa2_sb, wab_sb = weights0
                else:
                    W_sb, wa2_sb, wab_sb = load_layer_weights(l)

                # ---- f1 broadcast across partitions (gates strip 0) ----
                f1b = fvp.tile([128, 1, N], f16, tag="f1b")
                for ic in range(IC):
                    ps = psp.tile([128, 512], f32, tag="bank", name=f"f1b_{l}_{ic}")
                    for fc in range(FC):
                        nc.tensor.matmul(
                            ps[:, :],
                            wab_sb[:, fc * 128 : (fc + 1) * 128],
                            hT_cur[:, fc * N + ic * 512 : fc * N + (ic + 1) * 512],
                            start=(fc == 0),
                            stop=(fc == FC - 1),
                        )
                    nc.vector.tensor_copy(
                        f1b[:, 0, ic * 512 : (ic + 1) * 512], ps[:, :]
                    )

                # ---- f2, in quarters so strip 0 unblocks early ----
                f2_sb = fvp.tile([128, NT], f32, tag="f2")
                f2s_sb = fvp.tile([128, NT], f32, tag="f2s")
                ps_f2 = psp.tile([128, 512], f32, tag="bank", name=f"f2_{l}")
                for q in range(4):
                    for nt in range(q * 4, q * 4 + 4):
                        for fc in range(FC):
                            nc.tensor.matmul(
                                ps_f2[:, nt : nt + 1],
                                hT_cur[:, fc * N + nt * 128 : fc * N + (nt + 1) * 128],
                                wa2_sb[:, fc : fc + 1],
                                start=(fc == 0),
                                stop=(fc == FC - 1),
                            )
                    nc.vector.tensor_copy(
                        f2_sb[:, q * 4 : q * 4 + 4], ps_f2[:, q * 4 : q * 4 + 4]
                    )
                    # f2s = -0.8*f2: max(t, 0.2t + f2s) + f2 == lrelu(t + f2)
                    nc.vector.tensor_scalar(
                        f2s_sb[:, q * 4 : q * 4 + 4],
                        f2_sb[:, q * 4 : q * 4 + 4],
                        ALPHA - 1.0, None, ALU.mult,
                    )

                # ---- Wh (two node tiles share a PSUM bank and one copy) ----
                Wh_sb = whp.tile([128, NT * F], f16, tag="Wh")
                for np_ in range(NT // 2):
                    ps = psp.tile([128, 512], f32, tag="bank", name=f"wh_{l}_{np_}")
                    for half in range(2):
                        nt = np_ * 2 + half
                        for fc in range(FC):
                            nc.tensor.matmul(
                                ps[:, half * F : (half + 1) * F],
                                hT_cur[:, fc * N + nt * 128 : fc * N + (nt + 1) * 128],
                                W_sb[:, fc * F : (fc + 1) * F],
                                start=(fc == 0),
                                stop=(fc == FC - 1),
                            )
                    dst = Wh_sb[:, np_ * 512 : (np_ + 1) * 512]
                    if WHCP[np_] == "a":
                        nc.scalar.copy(dst, ps[:, :])
                    else:
                        nc.vector.tensor_copy(dst, ps[:, :])

                # ---- strip loop ----
                psum_out = [
                    psp.tile([128, 512], f32, tag="bank", name=f"po_{l}_{k}")
                    for k in range(8)
                ]
                t_pairs = [None] * (NT // 2)
                expe_t = [None] * NT
                s_t = [None] * (NT // 2)

                def emit_front(jt):
                    if jt % 2 == 0:
                        tp = tpp.tile([128, 2, N], f16, tag="t", name=f"t_{l}_{jt}")
                        t_pairs[jt // 2] = tp
                        nc.vector.tensor_tensor(
                            tp[:, :, :],
                            f1b[:, :, :].broadcast_to([128, 2, N]),
                            mask_sb[:, jt : jt + 2, :],
                            ALU.add,
                        )
                        s_t[jt // 2] = srp.tile(
                            [128, 2], f32, tag="s", name=f"s_{l}_{jt}"
                        )
                    t = t_pairs[jt // 2][:, jt % 2, :]
                    s_col = s_t[jt // 2][:, jt % 2 : jt % 2 + 1]
                    f2c = f2_sb[:, jt : jt + 1]
                    exp_bias = 0.0
                    if ROUTES[jt] == "A":
                        nc.scalar.activation(
                            t, t, AFT.Prelu, bias=f2c, scale=1.0, alpha=ALPHA
                        )
                    else:
                        w = zwp.tile([128, N], f16, tag="w", name=f"w_{l}_{jt}")
                        nc.vector.tensor_scalar(
                            w[:, :], t, ALPHA, f2s_sb[:, jt : jt + 1],
                            ALU.mult, ALU.add,
                        )
                        nc.vector.tensor_tensor(t, t, w[:, :], ALU.max)
                        exp_bias = f2c
                    expe = expp.tile([128, N], f16, tag="expe", name=f"ex_{l}_{jt}")
                    nc.scalar.activation(
                        expe[:, :], t, AFT.Exp, bias=exp_bias, accum_out=s_col
                    )
                    expe_t[jt] = expe

                def emit_back_pair(jt):
                    # strips jt, jt+1; r for both from one reciprocal
                    r = srp.tile([128, 2], f32, tag="r", name=f"r_{l}_{jt}")
                    nc.vector.reciprocal(r[:, :], s_t[jt // 2][:, :])
                    whs = [None, None]
                    for k in range(2):
                        whs[k] = whsp.tile([128, F], f16, tag="whs",
                                           name=f"whs_{l}_{jt + k}")
                        nc.vector.tensor_scalar_mul(
                            whs[k][:, :],
                            Wh_sb[:, (jt + k) * F : (jt + k + 1) * F],
                            r[:, k : k + 1],
                        )
                    # 16 consecutive matmuls keep the PE p-state ramped
                    for k in range(2):
                        for oc in range(2):
                            for ic in range(IC):
                                nc.tensor.matmul(
                                    psum_out[oc * IC + ic][:, :],
                                    whs[k][:, oc * 128 : (oc + 1) * 128],
                                    expe_t[jt + k][:, ic * 512 : (ic + 1) * 512],
                                    start=(jt + k == 0),
                                    stop=(jt + k == NT - 1),
                                )

                for step in range(NT + 2):
                    if step >= 2 and step % 2 == 0:
                        emit_back_pair(step - 2)
                    if step < NT:
                        emit_front(step)

                # ---- tail: prelu psum -> next hT (fp16), on ACT (bubble) ----
                hT_next = hTp.tile([128, FC * N], f16, tag="hT", name=f"hTn_{l}")
                for ic in range(IC):
                    for oc in range(2):
                        k = oc * IC + ic
                        dst = hT_next[:, oc * N + ic * 512 : oc * N + (ic + 1) * 512]
                        nc.scalar.activation(
                            dst, psum_out[k][:, :], AFT.Prelu, alpha=ALPHA
                        )
                if l < L - 1:
                    hT_cur = hT_next
                else:
                    for oc in range(FC):
                        nc.sync.dma_start(
                            outT_d[oc * 128 : (oc + 1) * 128, :],
                            hT_next[:, oc * N : (oc + 1) * N],
                        )

    if do_split:
        split_multi_waits(nc)
    return nc


_NC = None


def _get_nc():
    global _NC
    if _NC is None:
        _NC = build_nc()
    return _NC


def _host_prep(x, adj, W0, Wrest, A):
    x = np.asarray(x, dtype=np.float32)
    adj = np.asarray(adj)
    W_all = np.stack(
        [np.asarray(W0, dtype=np.float32)]
        + [np.asarray(Wrest[i], dtype=np.float32) for i in range(L - 1)]
    )  # [4, F, F]
    A = np.asarray(A, dtype=np.float32)
    wa1 = np.empty((L, F), dtype=np.float32)
    wa2 = np.empty((L, F, 1), dtype=np.float32)
    for l in range(L):
        wa1[l] = W_all[l] @ A[l, :F]
        wa2[l, :, 0] = W_all[l] @ A[l, F:]
    W_16 = W_all.astype(np.float16)
    wa2_16 = wa2.astype(np.float16)
    wab_16 = np.repeat(wa1.astype(np.float16)[:, :, None], 128, axis=2)

    in_maps = []
    for b in range(B):
        xT = np.ascontiguousarray(x[b].T).astype(np.float16)
        maskT = ((adj[b].T == 0) * np.float32(MASKADD)).astype(np.float16)
        in_maps.append(
            {
                "xT": xT,
                "maskT": maskT,
                "W": W_16,
                "wa2": wa2_16,
                "wab": wab_16,
            }
        )
    return in_maps


def kernel(x, adj, W0, Wrest, A, _trace=False, _trace_kwargs=None):
    nc = _get_nc()
    in_maps = _host_prep(x, adj, W0, Wrest, A)
    res = run_bass_kernel_spmd(
        nc,
        in_maps,
        core_ids=list(range(B)),
        trace=_trace,
        **(_trace_kwargs or {}),
    )
    out = np.stack(
        [
            np.ascontiguousarray(res.results[b]["outT"].astype(np.float32).T)
            for b in range(B)
        ]
    )
    if _trace:
        kernel.last_exec_time_ns = res.exec_time_ns
        kernel.last_results = res
    return out
